# revision 18
# baseline (speedup 1.0000x reference)
"""Bilateral blur (7x7, L1 color distance) on 8 Trainium2 NeuronCores, v2.

Input (4, 3, 512, 512) fp32 -> output (4, 3, 512, 512) fp32.

Sharding: core i handles batch i//2, row-half i%2 (256x512 px). Each of the
128 partitions owns a 32x32 output tile (16 col-groups x 8 row-slices) and
holds the matching 38x38 padded patch per channel in fp16 ([c][xl][yl],
yl contiguous), plus a 1-element-shifted copy so every tap read is 4-byte
aligned (DVE 2x fp16 mode).

Algorithm (per pair of symmetric taps +/-D, D=(dy,dx), 24 pairs + center):
  out = x + (sum_k w_k * dlt_k) / (sum_k w_k),  dlt_k = x(p+D_k) - x(p)
The weight field W(q) = s_D * exp(-50 * d(q)^2), d = sum_c |dlt_c|, is
SYMMETRIC: tap -D at pixel p uses W(p-D), and its numerator contribution is
-P(p-D) where P = W (*) dlt is the same product field used by tap +D. So
distance, exp and the multiply are computed ONCE per pair over a slightly
extended domain (EX=32+|dx|, EY=32+dy), and the PE accumulates both taps
via identity matmuls into PSUM: +I on [P|W](p-domain), -I on P and +I on W
at the mirrored offset. The center tap is exact: contributes only s0 to the
denominator (folded into the PSUM->SBUF copy as an ACT bias).

Engine split per pair (balanced ~4.7us/pair): DVE does the subtract, |dlt2|
(u32 sign-mask at 4 fp16/cyc), and the product (fp16 TT 2x); ACT does
|dlt0|,|dlt1| (one 2-channel Abs), Square and Exp (s_k rides the exp bias
as ln s_k); Pool (GPSIMD) does the first channel-sum add, and the second
add alternates DVE/Pool in a sweep-tuned pattern; PE does 16 accumulate
matmuls. Emission is software-pipelined in 3 stages so each in-order engine
queue always has ready work. The two drain pairs keep their whole chain on
DVE to fill its pipeline-drain gaps; the last pair closes the denominator
PSUM banks before its product mult so the reciprocal overlaps the final
matmuls; pair 0's subtract and the input DMAs are split per channel so
compute starts before the full patch lands; ACT tables are pre-warmed via a
dependency-free scale=0 activation.
"""
import numpy as np

import concourse.bass as bass
import concourse.bacc as bacc
import concourse.mybir as mybir
from concourse.tile import TileContext
from concourse import bass_utils

C = 3
B, H, W = 4, 512, 512
PAD = 3
SIGMA_COLOR = 0.1
N_CORES = 8

TS = 32                      # tile side (output px per partition: TS x TS)
PS = TS + 2 * PAD            # padded patch side = 38
NPART = 128
GX, RY = 16, 8               # col-groups x row-slices = 128 partitions
ROWS = RY * TS               # 256 output rows per core
CS = PS * PS                 # per-channel patch stride = 1444
FIN = C * CS                 # 4332
SX = 36                      # field xl stride (even, >= max EY=35)
CF = SX * 35                 # per-channel field stride = 1260 (even)
F = TS * TS                  # 1024
SCL = -0.5 / SIGMA_COLOR ** 2

# symmetric tap pairs: (dy, dx) with dy>0, or dy==0 and dx>0
_P0 = [(0, dx) for dx in range(1, 4)] + \
      [(dy, dx) for dy in range(1, 4) for dx in range(-3, 4)]
# small fields at pipeline fill/drain ends, big in the middle
_PS = sorted(_P0, key=lambda p: (32 + abs(p[1])) * (32 + p[0]))
PAIRS = _PS[:12][0::2] + _PS[12:] + _PS[:12][1::2][::-1]
# per-pair engine assignment knobs (k -> bool), tuned via TimelineSim sweeps
CH1_ACT = lambda k: True      # |dlt1| on ACT (with ch0) vs DVE u32
_N24 = len(_P0)
# d-add alternates DVE/GPSIMD; drain pairs keep their whole chain on DVE
DADD_DVE = lambda k: k % 5 in (0, 2) or k >= _N24 - 2
S1_DVE = lambda k: k >= _N24 - 2  # s1-add on DVE for the drain pairs
SUB2_POOL = lambda k: False   # ch2 of the subtract on GPSIMD (parallel lane)
SQ_DVE = lambda k: k >= _N24 - 2  # square on DVE for the drain pairs
ABS3_ACT = lambda k: False    # all 3 abs channels on ACT (no DVE u32 op)
GAUSS_DE = lambda k: False    # Derivative_Erf gaussian + DVE 4x s_k scale
HEAD_OPT = False              # ip/im DMAs on the sync queue
DEPTH4 = False                # 4-deep software pipeline (split stage2)
TAIL_OPT = True               # PSUM->SBUF copies before the reciprocal


def _g1(k, sigma):
    x = np.arange(k, dtype=np.float64) - (k - 1) / 2.0
    g = np.exp(-0.5 * (x / sigma) ** 2)
    return g / g.sum()


_G = _g1(7, 1.5)
_S_PAIR = np.array([_G[3 + dy] * _G[3 + dx] for (dy, dx) in PAIRS])
_S0 = float(_G[3] * _G[3])


def _build(ntaps=len(PAIRS)):
    nc = bacc.Bacc()
    f32 = mybir.dt.float32
    f16 = mybir.dt.float16
    xt = nc.dram_tensor("xt", [NPART, FIN], f16, kind="ExternalInput")
    lnsb = nc.dram_tensor("lnsb", [NPART, len(PAIRS) + 1], f32,
                          kind="ExternalInput")
    skt = nc.dram_tensor("skt", [NPART, len(PAIRS)], f32,
                         kind="ExternalInput")
    identp = nc.dram_tensor("identp", [NPART, NPART], f16, kind="ExternalInput")
    identn = nc.dram_tensor("identn", [NPART, NPART], f16, kind="ExternalInput")
    o = nc.dram_tensor("o", [NPART, C * F], f16, kind="ExternalOutput")
    AOT = mybir.AluOpType
    AFT = mybir.ActivationFunctionType

    with TileContext(nc) as tc:
        with tc.tile_pool(name="persist", bufs=1) as pool, \
             tc.tile_pool(name="tmpb", bufs=5) as tpb, \
             tc.tile_pool(name="tmp", bufs=4) as tp, \
             tc.tile_pool(name="ps", bufs=1, space="PSUM") as psp:
            # warm the ACT spline tables immediately (scale=0 makes the
            # input values irrelevant, so it reads its own uninitialized tile
            # and has no DMA dependency)
            warm = pool.tile([NPART, 2], f16, name="warm")
            nc.scalar.activation(warm[:], warm[:], AFT.Exp, bias=0.0,
                                 scale=0.0)
            bias = pool.tile([NPART, len(PAIRS) + 1], f32, name="bias")
            nc.sync.dma_start(bias[:], lnsb[:])
            sk = pool.tile([NPART, len(PAIRS)], f32, name="sk")
            nc.sync.dma_start(sk[:], skt[:])
            Te = pool.tile([NPART, FIN], f16, name="Te")
            for cc in range(C):
                nc.sync.dma_start(Te[:, cc * CS:(cc + 1) * CS],
                                  xt[:, cc * CS:(cc + 1) * CS])
            small_dma = nc.gpsimd.dma_start if HEAD_OPT else nc.sync.dma_start
            ip = pool.tile([NPART, NPART], f16, name="ip")
            small_dma(ip[:], identp[:])
            im = pool.tile([NPART, NPART], f16, name="im")
            small_dma(im[:], identn[:])
            # To[i] = patch[i+1]: shifts yl by one within each patch column so
            # odd patch y-offsets become 4B-aligned reads (DVE 2x fp16).
            # Loaded straight from DRAM so it overlaps the Te DMA.
            To = pool.tile([NPART, FIN], f16, name="To")
            xta = xt[:]
            for cc in range(C):
                hi = min((cc + 1) * CS, FIN - 1)
                nc.scalar.dma_start(
                    To[:, cc * CS:hi],
                    bass.AP(xta.tensor, xta.offset + cc * CS + 1,
                            [[FIN, NPART], [1, hi - cc * CS]]))
            acc = psp.tile([NPART, 4 * F], f32, name="acc")

            def patch_ap(dy_first, x0, y0, exx, ey, nch=C, ch0=0):
                """Read [c][xl: exx][yl: ey] at patch (x0, y0); picks the
                shifted copy so the element offset is even."""
                t, yy = (Te, y0) if y0 % 2 == 0 else (To, y0 - 1)
                a = t[:]
                dims = [[FIN, NPART], [CS, nch], [PS, exx], [1, ey]]
                if nch == 1:
                    dims = [dims[0]] + dims[2:]
                return bass.AP(a.tensor, a.offset + ch0 * CS + x0 * PS + yy,
                               dims)

            def fld(t, nch, exx, ey, ch0=0, bcast=False):
                a = t[:]
                dims = [[a.shape[1], NPART], [0 if bcast else CF, nch],
                        [SX, exx], [1, ey]]
                if nch == 1:
                    dims = [dims[0]] + dims[2:]
                return bass.AP(a.tensor, a.offset + ch0 * CF, dims)

            started = [False] * 8
            npairs = min(ntaps, len(PAIRS))
            live = {}

            def chunks(tt, x0, y0, wfield):
                a = tt[:]
                res = []
                for c in range(1 if wfield else C):
                    for xh in range(2):
                        base = c * CF + (x0 + 16 * xh) * SX + y0
                        bank = 6 + xh if wfield else 2 * c + xh
                        res.append((bank, bass.AP(
                            a.tensor, a.offset + base,
                            [[a.shape[1], NPART], [SX, 16], [1, TS]])))
                return res

            def mm(bank, mov, stat, stop):
                nc.tensor.matmul(acc[:, bank * 512:(bank + 1) * 512],
                                 stat[:], mov,
                                 start=not started[bank], stop=stop)
                started[bank] = True

            def stage1(k):
                dy, dx = PAIRS[k]
                EX, EY = TS + abs(dx), TS + dy
                xt0 = PAD + min(dx, 0)          # tap patch x base
                xc0 = PAD - max(dx, 0)          # center patch x base
                dlt = tpb.tile([NPART, C * CF], f16, name="dlt", tag="dlt")
                if SUB2_POOL(k):
                    nc.vector.tensor_tensor(
                        out=fld(dlt, 2, EX, EY),
                        in0=patch_ap(None, xt0, PAD, EX, EY, nch=2),
                        in1=patch_ap(None, xc0, PAD - dy, EX, EY, nch=2),
                        op=AOT.subtract)
                    nc.gpsimd.tensor_tensor(
                        out=fld(dlt, 1, EX, EY, ch0=2),
                        in0=patch_ap(None, xt0, PAD, EX, EY, nch=1, ch0=2),
                        in1=patch_ap(None, xc0, PAD - dy, EX, EY, nch=1, ch0=2),
                        op=AOT.subtract)
                elif k == 0:
                    # pair 0: split the sub per channel so it starts as soon
                    # as the per-channel input DMAs land
                    nc.vector.tensor_tensor(
                        out=fld(dlt, 2, EX, EY),
                        in0=patch_ap(None, xt0, PAD, EX, EY, nch=2),
                        in1=patch_ap(None, xc0, PAD - dy, EX, EY, nch=2),
                        op=AOT.subtract)
                    nc.vector.tensor_tensor(
                        out=fld(dlt, 1, EX, EY, ch0=2),
                        in0=patch_ap(None, xt0, PAD, EX, EY, nch=1, ch0=2),
                        in1=patch_ap(None, xc0, PAD - dy, EX, EY, nch=1, ch0=2),
                        op=AOT.subtract)
                else:
                    nc.vector.tensor_tensor(
                        out=fld(dlt, C, EX, EY),
                        in0=patch_ap(None, xt0, PAD, EX, EY),
                        in1=patch_ap(None, xc0, PAD - dy, EX, EY),
                        op=AOT.subtract)
                # d = |dlt0|+|dlt1|+|dlt2|: abs split ACT/DVE per-pair to
                # balance engine load (DVE path: u32 sign-mask at 4 fp16/cyc)
                nact = 3 if ABS3_ACT(k) else (2 if CH1_ACT(k) else 1)
                a0w = max(3 if ABS3_ACT(j) else 2 for j in range(npairs))
                a0 = tp.tile([NPART, a0w * CF], f16, name="a0", tag="a0")
                nc.scalar.activation(fld(a0, nact, EX, EY),
                                     fld(dlt, nact, EX, EY),
                                     AFT.Abs, bias=0.0, scale=1.0)
                adl = tp.tile([NPART, CF], f16, name="adl", tag="adl")
                if nact < 3:
                    nc.vector.tensor_scalar(
                        out=adl[:, 0:(3 - nact) * CF].bitcast(mybir.dt.uint32),
                        in0=dlt[:, nact * CF:3 * CF].bitcast(mybir.dt.uint32),
                        scalar1=0x7FFF7FFF, scalar2=None, op0=AOT.bitwise_and)
                live[k] = {"dlt": dlt, "a0": a0, "adl": adl, "nact": nact}

            def stage2a(k):
                dy, dx = PAIRS[k]
                EX, EY = TS + abs(dx), TS + dy
                t = live[k]
                nact = t["nact"]
                ch1s = t["a0"] if nact >= 2 else t["adl"]
                ch1o = 1 if nact >= 2 else 0
                s1 = tp.tile([NPART, CF], f16, name="s1", tag="s1")
                s1eng = nc.vector if S1_DVE(k) else nc.gpsimd
                s1eng.tensor_tensor(
                    out=fld(s1, 1, EX, EY), in0=fld(t["a0"], 1, EX, EY),
                    in1=fld(ch1s, 1, EX, EY, ch0=ch1o), op=AOT.add)
                t["s1"] = s1

            def stage2b(k):
                dy, dx = PAIRS[k]
                EX, EY = TS + abs(dx), TS + dy
                t = live[k]
                nact = t["nact"]
                ch2s = t["a0"] if nact == 3 else t["adl"]
                ch2o = 2 if nact == 3 else (0 if nact == 2 else 1)
                s1 = t["s1"]
                d = tp.tile([NPART, CF], f16, name="d", tag="d")
                dadd = nc.vector if DADD_DVE(k) else nc.gpsimd
                dadd.tensor_tensor(
                    out=fld(d, 1, EX, EY), in0=fld(s1, 1, EX, EY),
                    in1=fld(ch2s, 1, EX, EY, ch0=ch2o), op=AOT.add)
                _emit_qw(k, d, EX, EY, t)

            def stage2(k):
                dy, dx = PAIRS[k]
                EX, EY = TS + abs(dx), TS + dy
                t = live[k]
                # s1 = |dlt0|+|dlt1| on GPSIMD, d = s1+|dlt2| mostly on DVE
                nact = t["nact"]
                ch1s = t["a0"] if nact >= 2 else t["adl"]
                ch1o = 1 if nact >= 2 else 0
                ch2s = t["a0"] if nact == 3 else t["adl"]
                ch2o = 2 if nact == 3 else (0 if nact == 2 else 1)
                s1 = tp.tile([NPART, CF], f16, name="s1", tag="s1")
                s1eng = nc.vector if S1_DVE(k) else nc.gpsimd
                s1eng.tensor_tensor(
                    out=fld(s1, 1, EX, EY), in0=fld(t["a0"], 1, EX, EY),
                    in1=fld(ch1s, 1, EX, EY, ch0=ch1o), op=AOT.add)
                d = tp.tile([NPART, CF], f16, name="d", tag="d")
                dadd = nc.vector if DADD_DVE(k) else nc.gpsimd
                dadd.tensor_tensor(
                    out=fld(d, 1, EX, EY), in0=fld(s1, 1, EX, EY),
                    in1=fld(ch2s, 1, EX, EY, ch0=ch2o), op=AOT.add)
                _emit_qw(k, d, EX, EY, t)

            def _emit_qw(k, d, EX, EY, t):
                q = tp.tile([NPART, CF], f16, name="q", tag="q")
                w = tp.tile([NPART, CF], f16, name="w", tag="w")
                if GAUSS_DE(k):
                    # exp(-50 d^2) = (sqrt(pi)/2) * DerivErf(sqrt(50) d);
                    # s_k * sqrt(pi)/2 folded into a DVE 4x tensor_scalar
                    nc.scalar.activation(fld(q, 1, EX, EY), fld(d, 1, EX, EY),
                                         AFT.Derivative_Erf, bias=0.0,
                                         scale=(-SCL) ** 0.5)
                    nc.vector.tensor_scalar(
                        out=fld(w, 1, EX, EY), in0=fld(q, 1, EX, EY),
                        scalar1=sk[:, k:k + 1], scalar2=None, op0=AOT.mult)
                else:
                    if SQ_DVE(k):
                        nc.vector.tensor_tensor(
                            out=fld(q, 1, EX, EY), in0=fld(d, 1, EX, EY),
                            in1=fld(d, 1, EX, EY), op=AOT.mult)
                    else:
                        nc.scalar.activation(fld(q, 1, EX, EY),
                                             fld(d, 1, EX, EY),
                                             AFT.Square, bias=0.0, scale=1.0)
                    nc.scalar.activation(fld(w, 1, EX, EY), fld(q, 1, EX, EY),
                                         AFT.Exp, bias=bias[:, k:k + 1],
                                         scale=SCL)
                t["w"] = w

            def stage3(k):
                dy, dx = PAIRS[k]
                EX, EY = TS + abs(dx), TS + dy
                last = (k == npairs - 1)
                t = live.pop(k)
                w = t["w"]
                xa, xb = max(dx, 0), max(-dx, 0)
                if last:
                    # den banks close before the product mult even runs, so
                    # the den copy + reciprocal overlap the final matmuls
                    for bank, mov in chunks(w, xa, dy, True):
                        mm(bank, mov, ip, False)
                    for bank, mov in chunks(w, xb, 0, True):
                        mm(bank, mov, ip, True)
                P = tpb.tile([NPART, C * CF], f16, name="P", tag="P")
                nc.vector.tensor_tensor(
                    out=fld(P, C, EX, EY), in0=fld(t["dlt"], C, EX, EY),
                    in1=fld(w, C, EX, EY, bcast=True), op=AOT.mult)

                # PE accumulate. Pass A (+I): [P|w] at field base (max(dx,0), dy)
                # covers tap +D; pass B at base (max(-dx,0), 0) covers tap -D:
                # +I on w, -I on P.
                if last:
                    # w matmuls were emitted before the mult; stop each num
                    # bank as early as possible so the copies start sooner
                    for (bank, mova), (_, movb) in zip(
                            chunks(P, xa, dy, False), chunks(P, xb, 0, False)):
                        mm(bank, mova, ip, False)
                        mm(bank, movb, im, True)
                else:
                    for bank, mov in chunks(P, xa, dy, False):
                        mm(bank, mov, ip, False)
                    for bank, mov in chunks(w, xa, dy, True):
                        mm(bank, mov, ip, False)
                    for bank, mov in chunks(w, xb, 0, True):
                        mm(bank, mov, ip, False)
                    for bank, mov in chunks(P, xb, 0, False):
                        mm(bank, mov, im, False)

            # software pipeline: each engine's in-order queue sees work for
            # pair k+1/k+2 before the cross-engine chain of pair k resolves
            if DEPTH4:
                for kk in range(npairs + 3):
                    if kk < npairs:
                        stage1(kk)
                    if 0 <= kk - 1 < npairs:
                        stage2a(kk - 1)
                    if 0 <= kk - 2 < npairs:
                        stage2b(kk - 2)
                    if 0 <= kk - 3 < npairs:
                        stage3(kk - 3)
            else:
                for kk in range(npairs + 2):
                    if kk < npairs:
                        stage1(kk)
                    if 0 <= kk - 1 < npairs:
                        stage2(kk - 1)
                    if 0 <= kk - 2 < npairs:
                        stage3(kk - 2)

            # out = x + num * (1/(den + s0)); per-channel for ACT/DVE overlap
            den = pool.tile([NPART, F], f32, name="den")
            for hh in range(2):
                nc.scalar.activation(den[:, hh * 512:(hh + 1) * 512],
                                     acc[:, 3 * F + hh * 512:3 * F + (hh + 1) * 512],
                                     AFT.Identity,
                                     bias=bias[:, len(PAIRS):len(PAIRS) + 1],
                                     scale=1.0)
            rc = pool.tile([NPART, F], f16, name="rc")
            nsb = pool.tile([NPART, C * F], f16, name="nsb")
            t16 = pool.tile([NPART, C * F], f16, name="t16")
            o16 = pool.tile([NPART, C * F], f16, name="o16")
            def copy_c(c):
                nc.scalar.activation(nsb[:, c * F:(c + 1) * F],
                                     acc[:, c * F:(c + 1) * F], AFT.Copy,
                                     bias=0.0, scale=1.0)
            if TAIL_OPT:
                for c in range(C):
                    copy_c(c)
            with nc.allow_low_precision(reason="fp16 out within 2e-2 budget"):
                for hh in range(2):
                    nc.vector.reciprocal(rc[:, hh * 512:(hh + 1) * 512],
                                         den[:, hh * 512:(hh + 1) * 512])
            rca = rc[:]
            for c in range(C):
                if not TAIL_OPT:
                    copy_c(c)
                nc.vector.tensor_tensor(
                    out=t16[:, c * F:(c + 1) * F],
                    in0=nsb[:, c * F:(c + 1) * F],
                    in1=bass.AP(rca.tensor, rca.offset, [[F, NPART], [1, F]]),
                    op=AOT.mult)
                t16a, o16a = t16[:], o16[:]
                nc.vector.tensor_tensor(
                    out=bass.AP(o16a.tensor, o16a.offset + c * F,
                                [[C * F, NPART], [TS, TS], [1, TS]]),
                    in0=bass.AP(t16a.tensor, t16a.offset + c * F,
                                [[C * F, NPART], [TS, TS], [1, TS]]),
                    in1=patch_ap(None, PAD, PAD, TS, TS, nch=1, ch0=c),
                    op=AOT.add)
                nc.sync.dma_start(o[:, c * F:(c + 1) * F],
                                  o16[:, c * F:(c + 1) * F])
    return nc


_LNSB = np.broadcast_to(
    np.concatenate([np.log(_S_PAIR), [_S0]]).astype(np.float32)[None, :],
    (NPART, len(PAIRS) + 1)).copy()
_SKT = np.broadcast_to(
    (_S_PAIR * np.sqrt(np.pi) / 2).astype(np.float32)[None, :],
    (NPART, len(PAIRS))).copy()

_NC_CACHE = {}


def _get_nc():
    if "nc" not in _NC_CACHE:
        nc = _build()
        nc.finalize()
        _NC_CACHE["nc"] = nc
    return _NC_CACHE["nc"]


def make_in_maps(x):
    xp = np.pad(x, ((0, 0), (0, 0), (PAD, PAD), (PAD, PAD)), mode="reflect")
    in_maps = []
    eye = np.eye(NPART, dtype=np.float16)
    for core in range(N_CORES):
        b, half = core // 2, core % 2
        r0 = half * ROWS
        shard = xp[b, :, r0:r0 + ROWS + 2 * PAD, :]       # (3, 262, 518)
        st = np.ascontiguousarray(shard.transpose(0, 2, 1))  # (3, 518, 262)
        se = st.strides
        v = np.lib.stride_tricks.as_strided(
            st,
            shape=(RY, GX, C, PS, PS),
            strides=(TS * se[2], TS * se[1], se[0], se[1], se[2]))
        buf = np.ascontiguousarray(v).astype(np.float16).reshape(NPART, FIN)
        in_maps.append({"xt": buf, "lnsb": _LNSB, "skt": _SKT,
                        "identp": eye, "identn": -eye})
    return in_maps


def kernel(input: np.ndarray) -> np.ndarray:
    x = np.asarray(input, dtype=np.float32)
    assert x.shape == (B, C, H, W)
    in_maps = make_in_maps(x)
    nc = _get_nc()
    res = bass_utils.run_bass_kernel_spmd(nc, in_maps, list(range(N_CORES)))
    out = np.empty((B, C, H, W), np.float32)
    for core in range(N_CORES):
        b, half = core // 2, core % 2
        r0 = half * ROWS
        ov = np.asarray(res.results[core]["o"]).astype(np.float32)
        ov = ov.reshape(RY, GX, C, TS, TS)              # (ry, gx, c, xf, yf)
        ov = ov.transpose(2, 0, 4, 1, 3).reshape(C, ROWS, W)
        out[b, :, r0:r0 + ROWS, :] = ov
    return out


# revision 19
# speedup vs baseline: 1.0080x; 1.0080x over previous
"""Bilateral blur (7x7, L1 color distance) on 8 Trainium2 NeuronCores, v2.

Input (4, 3, 512, 512) fp32 -> output (4, 3, 512, 512) fp32.

Sharding: core i handles batch i//2, row-half i%2 (256x512 px). Each of the
128 partitions owns a 32x32 output tile (16 col-groups x 8 row-slices) and
holds the matching 38x38 padded patch per channel in fp16 ([c][xl][yl],
yl contiguous), plus a 1-element-shifted copy so every tap read is 4-byte
aligned (DVE 2x fp16 mode).

Algorithm (per pair of symmetric taps +/-D, D=(dy,dx), 24 pairs + center):
  out = x + (sum_k w_k * dlt_k) / (sum_k w_k),  dlt_k = x(p+D_k) - x(p)
The weight field W(q) = s_D * exp(-50 * d(q)^2), d = sum_c |dlt_c|, is
SYMMETRIC: tap -D at pixel p uses W(p-D), and its numerator contribution is
-P(p-D) where P = W (*) dlt is the same product field used by tap +D. So
distance, exp and the multiply are computed ONCE per pair over a slightly
extended domain (EX=32+|dx|, EY=32+dy), and the PE accumulates both taps
via identity matmuls into PSUM: +I on [P|W](p-domain), -I on P and +I on W
at the mirrored offset. The center tap is exact: contributes only s0 to the
denominator (folded into the PSUM->SBUF copy as an ACT bias).

Engine split per pair (balanced ~4.7us/pair): DVE does the subtract, |dlt2|
(u32 sign-mask at 4 fp16/cyc), and the product (fp16 TT 2x); ACT does
|dlt0|,|dlt1| (one 2-channel Abs), Square and Exp (s_k rides the exp bias
as ln s_k); Pool (GPSIMD) does the first channel-sum add, and the second
add alternates DVE/Pool in a sweep-tuned pattern; PE does 16 accumulate
matmuls. Emission is software-pipelined in 3 stages so each in-order engine
queue always has ready work. The two drain pairs keep their whole chain on
DVE to fill its pipeline-drain gaps; the last pair closes the denominator
PSUM banks before its product mult so the reciprocal overlaps the final
matmuls; pair 0's subtract and the input DMAs are split per channel so
compute starts before the full patch lands; ACT tables are pre-warmed via a
dependency-free scale=0 activation.
"""
import numpy as np

import concourse.bass as bass
import concourse.bacc as bacc
import concourse.mybir as mybir
from concourse.tile import TileContext
from concourse import bass_utils

C = 3
B, H, W = 4, 512, 512
PAD = 3
SIGMA_COLOR = 0.1
N_CORES = 8

TS = 32                      # tile side (output px per partition: TS x TS)
PS = TS + 2 * PAD            # padded patch side = 38
NPART = 128
GX, RY = 16, 8               # col-groups x row-slices = 128 partitions
ROWS = RY * TS               # 256 output rows per core
CS = PS * PS                 # per-channel patch stride = 1444
FIN = C * CS                 # 4332
SX = 36                      # field xl stride (even, >= max EY=35)
CF = SX * 35                 # per-channel field stride = 1260 (even)
F = TS * TS                  # 1024
SCL = -0.5 / SIGMA_COLOR ** 2

# symmetric tap pairs: (dy, dx) with dy>0, or dy==0 and dx>0
_P0 = [(0, dx) for dx in range(1, 4)] + \
      [(dy, dx) for dy in range(1, 4) for dx in range(-3, 4)]
# small fields at pipeline fill/drain ends, big in the middle
_PS = sorted(_P0, key=lambda p: (32 + abs(p[1])) * (32 + p[0]))
PAIRS = _PS[:12][0::2] + _PS[12:] + _PS[:12][1::2][::-1]
# per-pair engine assignment knobs (k -> bool), tuned via TimelineSim sweeps
CH1_ACT = lambda k: True      # |dlt1| on ACT (with ch0) vs DVE u32
_N24 = len(_P0)
# d-add alternates DVE/GPSIMD; drain pairs keep their whole chain on DVE
DADD_DVE = lambda k: k % 5 in (0, 2) or k >= _N24 - 2
S1_DVE = lambda k: k >= _N24 - 2  # s1-add on DVE for the drain pairs
SUB2_POOL = lambda k: False   # ch2 of the subtract on GPSIMD (parallel lane)
SQ_DVE = lambda k: k >= _N24 - 2  # square on DVE for the drain pairs
ABS3_ACT = lambda k: False    # all 3 abs channels on ACT (no DVE u32 op)
GAUSS_DE = lambda k: False    # Derivative_Erf gaussian + DVE 4x s_k scale
HEAD_OPT = False              # ip/im DMAs on the sync queue
DEPTH4 = False                # 4-deep software pipeline (split stage2)
TAIL_OPT = True               # PSUM->SBUF copies before the reciprocal


def _g1(k, sigma):
    x = np.arange(k, dtype=np.float64) - (k - 1) / 2.0
    g = np.exp(-0.5 * (x / sigma) ** 2)
    return g / g.sum()


_G = _g1(7, 1.5)
_S_PAIR = np.array([_G[3 + dy] * _G[3 + dx] for (dy, dx) in PAIRS])
_S0 = float(_G[3] * _G[3])


def _build(ntaps=len(PAIRS)):
    nc = bacc.Bacc()
    f32 = mybir.dt.float32
    f16 = mybir.dt.float16
    xt = nc.dram_tensor("xt", [NPART, FIN], f16, kind="ExternalInput")
    lnsb = nc.dram_tensor("lnsb", [NPART, len(PAIRS) + 1], f32,
                          kind="ExternalInput")
    skt = nc.dram_tensor("skt", [NPART, len(PAIRS)], f32,
                         kind="ExternalInput")
    identp = nc.dram_tensor("identp", [NPART, NPART], f16, kind="ExternalInput")
    identn = nc.dram_tensor("identn", [NPART, NPART], f16, kind="ExternalInput")
    o = nc.dram_tensor("o", [NPART, C * F], f16, kind="ExternalOutput")
    AOT = mybir.AluOpType
    AFT = mybir.ActivationFunctionType

    with TileContext(nc) as tc:
        with tc.tile_pool(name="persist", bufs=1) as pool, \
             tc.tile_pool(name="tmpb", bufs=5) as tpb, \
             tc.tile_pool(name="tmp", bufs=4) as tp, \
             tc.tile_pool(name="ps", bufs=1, space="PSUM") as psp:
            # warm the ACT spline tables immediately (scale=0 makes the
            # input values irrelevant, so it reads its own uninitialized tile
            # and has no DMA dependency)
            warm = pool.tile([NPART, 2], f16, name="warm")
            nc.scalar.activation(warm[:], warm[:], AFT.Exp, bias=0.0,
                                 scale=0.0)
            bias = pool.tile([NPART, len(PAIRS) + 1], f32, name="bias")
            nc.sync.dma_start(bias[:], lnsb[:])
            sk = pool.tile([NPART, len(PAIRS)], f32, name="sk")
            nc.sync.dma_start(sk[:], skt[:])
            Te = pool.tile([NPART, FIN], f16, name="Te")
            for cc in range(C):
                nc.sync.dma_start(Te[:, cc * CS:(cc + 1) * CS],
                                  xt[:, cc * CS:(cc + 1) * CS])
            small_dma = nc.gpsimd.dma_start if HEAD_OPT else nc.sync.dma_start
            ip = pool.tile([NPART, NPART], f16, name="ip")
            small_dma(ip[:], identp[:])
            im = pool.tile([NPART, NPART], f16, name="im")
            small_dma(im[:], identn[:])
            # To[i] = patch[i+1]: shifts yl by one within each patch column so
            # odd patch y-offsets become 4B-aligned reads (DVE 2x fp16).
            # Loaded straight from DRAM so it overlaps the Te DMA.
            To = pool.tile([NPART, FIN], f16, name="To")
            xta = xt[:]
            for cc in range(C):
                hi = min((cc + 1) * CS, FIN - 1)
                nc.scalar.dma_start(
                    To[:, cc * CS:hi],
                    bass.AP(xta.tensor, xta.offset + cc * CS + 1,
                            [[FIN, NPART], [1, hi - cc * CS]]))
            acc = psp.tile([NPART, 4 * F], f32, name="acc")

            def patch_ap(dy_first, x0, y0, exx, ey, nch=C, ch0=0):
                """Read [c][xl: exx][yl: ey] at patch (x0, y0); picks the
                shifted copy so the element offset is even."""
                t, yy = (Te, y0) if y0 % 2 == 0 else (To, y0 - 1)
                a = t[:]
                dims = [[FIN, NPART], [CS, nch], [PS, exx], [1, ey]]
                if nch == 1:
                    dims = [dims[0]] + dims[2:]
                return bass.AP(a.tensor, a.offset + ch0 * CS + x0 * PS + yy,
                               dims)

            def fld(t, nch, exx, ey, ch0=0, bcast=False):
                a = t[:]
                dims = [[a.shape[1], NPART], [0 if bcast else CF, nch],
                        [SX, exx], [1, ey]]
                if nch == 1:
                    dims = [dims[0]] + dims[2:]
                return bass.AP(a.tensor, a.offset + ch0 * CF, dims)

            started = [False] * 8
            npairs = min(ntaps, len(PAIRS))
            live = {}
            tailw = {}

            def chunks(tt, x0, y0, wfield):
                a = tt[:]
                res = []
                for c in range(1 if wfield else C):
                    for xh in range(2):
                        base = c * CF + (x0 + 16 * xh) * SX + y0
                        bank = 6 + xh if wfield else 2 * c + xh
                        res.append((bank, bass.AP(
                            a.tensor, a.offset + base,
                            [[a.shape[1], NPART], [SX, 16], [1, TS]])))
                return res

            def mm(bank, mov, stat, stop):
                nc.tensor.matmul(acc[:, bank * 512:(bank + 1) * 512],
                                 stat[:], mov,
                                 start=not started[bank], stop=stop)
                started[bank] = True

            def stage1(k):
                dy, dx = PAIRS[k]
                EX, EY = TS + abs(dx), TS + dy
                xt0 = PAD + min(dx, 0)          # tap patch x base
                xc0 = PAD - max(dx, 0)          # center patch x base
                dlt = tpb.tile([NPART, C * CF], f16, name="dlt", tag="dlt")
                if SUB2_POOL(k):
                    nc.vector.tensor_tensor(
                        out=fld(dlt, 2, EX, EY),
                        in0=patch_ap(None, xt0, PAD, EX, EY, nch=2),
                        in1=patch_ap(None, xc0, PAD - dy, EX, EY, nch=2),
                        op=AOT.subtract)
                    nc.gpsimd.tensor_tensor(
                        out=fld(dlt, 1, EX, EY, ch0=2),
                        in0=patch_ap(None, xt0, PAD, EX, EY, nch=1, ch0=2),
                        in1=patch_ap(None, xc0, PAD - dy, EX, EY, nch=1, ch0=2),
                        op=AOT.subtract)
                elif k == 0:
                    # pair 0: split the sub per channel so it starts as soon
                    # as the per-channel input DMAs land
                    nc.vector.tensor_tensor(
                        out=fld(dlt, 2, EX, EY),
                        in0=patch_ap(None, xt0, PAD, EX, EY, nch=2),
                        in1=patch_ap(None, xc0, PAD - dy, EX, EY, nch=2),
                        op=AOT.subtract)
                    nc.vector.tensor_tensor(
                        out=fld(dlt, 1, EX, EY, ch0=2),
                        in0=patch_ap(None, xt0, PAD, EX, EY, nch=1, ch0=2),
                        in1=patch_ap(None, xc0, PAD - dy, EX, EY, nch=1, ch0=2),
                        op=AOT.subtract)
                else:
                    nc.vector.tensor_tensor(
                        out=fld(dlt, C, EX, EY),
                        in0=patch_ap(None, xt0, PAD, EX, EY),
                        in1=patch_ap(None, xc0, PAD - dy, EX, EY),
                        op=AOT.subtract)
                # d = |dlt0|+|dlt1|+|dlt2|: abs split ACT/DVE per-pair to
                # balance engine load (DVE path: u32 sign-mask at 4 fp16/cyc)
                nact = 3 if ABS3_ACT(k) else (2 if CH1_ACT(k) else 1)
                a0w = max(3 if ABS3_ACT(j) else 2 for j in range(npairs))
                a0 = tp.tile([NPART, a0w * CF], f16, name="a0", tag="a0")
                nc.scalar.activation(fld(a0, nact, EX, EY),
                                     fld(dlt, nact, EX, EY),
                                     AFT.Abs, bias=0.0, scale=1.0)
                adl = tp.tile([NPART, CF], f16, name="adl", tag="adl")
                if nact < 3:
                    nc.vector.tensor_scalar(
                        out=adl[:, 0:(3 - nact) * CF].bitcast(mybir.dt.uint32),
                        in0=dlt[:, nact * CF:3 * CF].bitcast(mybir.dt.uint32),
                        scalar1=0x7FFF7FFF, scalar2=None, op0=AOT.bitwise_and)
                live[k] = {"dlt": dlt, "a0": a0, "adl": adl, "nact": nact}

            def stage2a(k):
                dy, dx = PAIRS[k]
                EX, EY = TS + abs(dx), TS + dy
                t = live[k]
                nact = t["nact"]
                ch1s = t["a0"] if nact >= 2 else t["adl"]
                ch1o = 1 if nact >= 2 else 0
                s1 = tp.tile([NPART, CF], f16, name="s1", tag="s1")
                s1eng = nc.vector if S1_DVE(k) else nc.gpsimd
                s1eng.tensor_tensor(
                    out=fld(s1, 1, EX, EY), in0=fld(t["a0"], 1, EX, EY),
                    in1=fld(ch1s, 1, EX, EY, ch0=ch1o), op=AOT.add)
                t["s1"] = s1

            def stage2b(k):
                dy, dx = PAIRS[k]
                EX, EY = TS + abs(dx), TS + dy
                t = live[k]
                nact = t["nact"]
                ch2s = t["a0"] if nact == 3 else t["adl"]
                ch2o = 2 if nact == 3 else (0 if nact == 2 else 1)
                s1 = t["s1"]
                d = tp.tile([NPART, CF], f16, name="d", tag="d")
                dadd = nc.vector if DADD_DVE(k) else nc.gpsimd
                dadd.tensor_tensor(
                    out=fld(d, 1, EX, EY), in0=fld(s1, 1, EX, EY),
                    in1=fld(ch2s, 1, EX, EY, ch0=ch2o), op=AOT.add)
                _emit_qw(k, d, EX, EY, t)

            def stage2(k):
                dy, dx = PAIRS[k]
                EX, EY = TS + abs(dx), TS + dy
                t = live[k]
                # s1 = |dlt0|+|dlt1| on GPSIMD, d = s1+|dlt2| mostly on DVE
                nact = t["nact"]
                ch1s = t["a0"] if nact >= 2 else t["adl"]
                ch1o = 1 if nact >= 2 else 0
                ch2s = t["a0"] if nact == 3 else t["adl"]
                ch2o = 2 if nact == 3 else (0 if nact == 2 else 1)
                s1 = tp.tile([NPART, CF], f16, name="s1", tag="s1")
                s1eng = nc.vector if S1_DVE(k) else nc.gpsimd
                s1eng.tensor_tensor(
                    out=fld(s1, 1, EX, EY), in0=fld(t["a0"], 1, EX, EY),
                    in1=fld(ch1s, 1, EX, EY, ch0=ch1o), op=AOT.add)
                d = tp.tile([NPART, CF], f16, name="d", tag="d")
                dadd = nc.vector if DADD_DVE(k) else nc.gpsimd
                dadd.tensor_tensor(
                    out=fld(d, 1, EX, EY), in0=fld(s1, 1, EX, EY),
                    in1=fld(ch2s, 1, EX, EY, ch0=ch2o), op=AOT.add)
                _emit_qw(k, d, EX, EY, t)

            def _emit_qw(k, d, EX, EY, t):
                q = tp.tile([NPART, CF], f16, name="q", tag="q")
                w = tp.tile([NPART, CF], f16, name="w", tag="w")
                if GAUSS_DE(k):
                    # exp(-50 d^2) = (sqrt(pi)/2) * DerivErf(sqrt(50) d);
                    # s_k * sqrt(pi)/2 folded into a DVE 4x tensor_scalar
                    nc.scalar.activation(fld(q, 1, EX, EY), fld(d, 1, EX, EY),
                                         AFT.Derivative_Erf, bias=0.0,
                                         scale=(-SCL) ** 0.5)
                    nc.vector.tensor_scalar(
                        out=fld(w, 1, EX, EY), in0=fld(q, 1, EX, EY),
                        scalar1=sk[:, k:k + 1], scalar2=None, op0=AOT.mult)
                else:
                    if SQ_DVE(k):
                        nc.vector.tensor_tensor(
                            out=fld(q, 1, EX, EY), in0=fld(d, 1, EX, EY),
                            in1=fld(d, 1, EX, EY), op=AOT.mult)
                    else:
                        nc.scalar.activation(fld(q, 1, EX, EY),
                                             fld(d, 1, EX, EY),
                                             AFT.Square, bias=0.0, scale=1.0)
                    nc.scalar.activation(fld(w, 1, EX, EY), fld(q, 1, EX, EY),
                                         AFT.Exp, bias=bias[:, k:k + 1],
                                         scale=SCL)
                t["w"] = w

            def stage3(k):
                dy, dx = PAIRS[k]
                EX, EY = TS + abs(dx), TS + dy
                last = (k == npairs - 1)
                t = live.pop(k)
                w = t["w"]
                xa, xb = max(dx, 0), max(-dx, 0)
                if last and npairs > 1:
                    # the last pair's W joins den via DVE in the tail; banks
                    # 6/7 already stopped at the previous pair, so the den
                    # copy overlaps this whole pair's compute
                    tailw.update(w=w, xa=xa, xb=xb, dy=dy)
                elif last:
                    for bank, mov in chunks(w, xa, dy, True):
                        mm(bank, mov, ip, False)
                    for bank, mov in chunks(w, xb, 0, True):
                        mm(bank, mov, ip, True)
                P = tpb.tile([NPART, C * CF], f16, name="P", tag="P")
                nc.vector.tensor_tensor(
                    out=fld(P, C, EX, EY), in0=fld(t["dlt"], C, EX, EY),
                    in1=fld(w, C, EX, EY, bcast=True), op=AOT.mult)

                # PE accumulate. Pass A (+I): [P|w] at field base (max(dx,0), dy)
                # covers tap +D; pass B at base (max(-dx,0), 0) covers tap -D:
                # +I on w, -I on P.
                if last:
                    # w matmuls were emitted before the mult; stop each num
                    # bank as early as possible so the copies start sooner
                    for (bank, mova), (_, movb) in zip(
                            chunks(P, xa, dy, False), chunks(P, xb, 0, False)):
                        mm(bank, mova, ip, False)
                        mm(bank, movb, im, True)
                else:
                    wstop = (k == npairs - 2)
                    for bank, mov in chunks(P, xa, dy, False):
                        mm(bank, mov, ip, False)
                    for bank, mov in chunks(w, xa, dy, True):
                        mm(bank, mov, ip, False)
                    for bank, mov in chunks(w, xb, 0, True):
                        mm(bank, mov, ip, wstop)
                    for bank, mov in chunks(P, xb, 0, False):
                        mm(bank, mov, im, False)

            # software pipeline: each engine's in-order queue sees work for
            # pair k+1/k+2 before the cross-engine chain of pair k resolves
            if DEPTH4:
                for kk in range(npairs + 3):
                    if kk < npairs:
                        stage1(kk)
                    if 0 <= kk - 1 < npairs:
                        stage2a(kk - 1)
                    if 0 <= kk - 2 < npairs:
                        stage2b(kk - 2)
                    if 0 <= kk - 3 < npairs:
                        stage3(kk - 3)
            else:
                for kk in range(npairs + 2):
                    if kk < npairs:
                        stage1(kk)
                    if 0 <= kk - 1 < npairs:
                        stage2(kk - 1)
                    if 0 <= kk - 2 < npairs:
                        stage3(kk - 2)

            # out = x + num * (1/(den + s0)); per-channel for ACT/DVE overlap
            den = pool.tile([NPART, F], f32, name="den")
            for hh in range(2):
                nc.scalar.activation(den[:, hh * 512:(hh + 1) * 512],
                                     acc[:, 3 * F + hh * 512:3 * F + (hh + 1) * 512],
                                     AFT.Identity,
                                     bias=bias[:, len(PAIRS):len(PAIRS) + 1],
                                     scale=1.0)
            rc = pool.tile([NPART, F], f16, name="rc")
            if tailw:
                wv = tailw["w"][:]
                d23 = pool.tile([NPART, F], f16, name="d23")
                den2 = pool.tile([NPART, F], f32, name="den2")

                def wap(x0, y0, hh):
                    return bass.AP(wv.tensor,
                                   wv.offset + (x0 + 16 * hh) * SX + y0,
                                   [[CF, NPART], [SX, 16], [1, TS]])
                d23a = d23[:]
                for hh in range(2):
                    nc.vector.tensor_tensor(
                        out=bass.AP(d23a.tensor, d23a.offset + hh * 512,
                                    [[F, NPART], [TS, 16], [1, TS]]),
                        in0=wap(tailw["xa"], tailw["dy"], hh),
                        in1=wap(tailw["xb"], 0, hh), op=AOT.add)
                    nc.vector.tensor_tensor(
                        out=den2[:, hh * 512:(hh + 1) * 512],
                        in0=den[:, hh * 512:(hh + 1) * 512],
                        in1=d23[:, hh * 512:(hh + 1) * 512],
                        op=AOT.add)
            nsb = pool.tile([NPART, C * F], f16, name="nsb")
            t16 = pool.tile([NPART, C * F], f16, name="t16")
            o16 = pool.tile([NPART, C * F], f16, name="o16")
            def copy_c(c):
                nc.scalar.activation(nsb[:, c * F:(c + 1) * F],
                                     acc[:, c * F:(c + 1) * F], AFT.Copy,
                                     bias=0.0, scale=1.0)
            if TAIL_OPT:
                for c in range(C):
                    copy_c(c)
            with nc.allow_low_precision(reason="fp16 out within 2e-2 budget"):
                for hh in range(2):
                    nc.vector.reciprocal(
                        rc[:, hh * 512:(hh + 1) * 512],
                        (den2 if tailw else den)[:, hh * 512:(hh + 1) * 512])
            rca = rc[:]
            for c in range(C):
                if not TAIL_OPT:
                    copy_c(c)
                nc.vector.tensor_tensor(
                    out=t16[:, c * F:(c + 1) * F],
                    in0=nsb[:, c * F:(c + 1) * F],
                    in1=bass.AP(rca.tensor, rca.offset, [[F, NPART], [1, F]]),
                    op=AOT.mult)
                t16a, o16a = t16[:], o16[:]
                nc.vector.tensor_tensor(
                    out=bass.AP(o16a.tensor, o16a.offset + c * F,
                                [[C * F, NPART], [TS, TS], [1, TS]]),
                    in0=bass.AP(t16a.tensor, t16a.offset + c * F,
                                [[C * F, NPART], [TS, TS], [1, TS]]),
                    in1=patch_ap(None, PAD, PAD, TS, TS, nch=1, ch0=c),
                    op=AOT.add)
                nc.sync.dma_start(o[:, c * F:(c + 1) * F],
                                  o16[:, c * F:(c + 1) * F])
    return nc


_LNSB = np.broadcast_to(
    np.concatenate([np.log(_S_PAIR), [_S0]]).astype(np.float32)[None, :],
    (NPART, len(PAIRS) + 1)).copy()
_SKT = np.broadcast_to(
    (_S_PAIR * np.sqrt(np.pi) / 2).astype(np.float32)[None, :],
    (NPART, len(PAIRS))).copy()

_NC_CACHE = {}


def _get_nc():
    if "nc" not in _NC_CACHE:
        nc = _build()
        nc.finalize()
        _NC_CACHE["nc"] = nc
    return _NC_CACHE["nc"]


def make_in_maps(x):
    xp = np.pad(x, ((0, 0), (0, 0), (PAD, PAD), (PAD, PAD)), mode="reflect")
    in_maps = []
    eye = np.eye(NPART, dtype=np.float16)
    for core in range(N_CORES):
        b, half = core // 2, core % 2
        r0 = half * ROWS
        shard = xp[b, :, r0:r0 + ROWS + 2 * PAD, :]       # (3, 262, 518)
        st = np.ascontiguousarray(shard.transpose(0, 2, 1))  # (3, 518, 262)
        se = st.strides
        v = np.lib.stride_tricks.as_strided(
            st,
            shape=(RY, GX, C, PS, PS),
            strides=(TS * se[2], TS * se[1], se[0], se[1], se[2]))
        buf = np.ascontiguousarray(v).astype(np.float16).reshape(NPART, FIN)
        in_maps.append({"xt": buf, "lnsb": _LNSB, "skt": _SKT,
                        "identp": eye, "identn": -eye})
    return in_maps


def kernel(input: np.ndarray) -> np.ndarray:
    x = np.asarray(input, dtype=np.float32)
    assert x.shape == (B, C, H, W)
    in_maps = make_in_maps(x)
    nc = _get_nc()
    res = bass_utils.run_bass_kernel_spmd(nc, in_maps, list(range(N_CORES)))
    out = np.empty((B, C, H, W), np.float32)
    for core in range(N_CORES):
        b, half = core // 2, core % 2
        r0 = half * ROWS
        ov = np.asarray(res.results[core]["o"]).astype(np.float32)
        ov = ov.reshape(RY, GX, C, TS, TS)              # (ry, gx, c, xf, yf)
        ov = ov.transpose(2, 0, 4, 1, 3).reshape(C, ROWS, W)
        out[b, :, r0:r0 + ROWS, :] = ov
    return out
